# revision 10
# baseline (speedup 1.0000x reference)
"""MobilityGNNLayer Trainium2 kernel (8 NeuronCores, SPMD, no collectives).

Sharding: 1D partition of the destination axis (columns of mobility_matrix).
Core c owns destination nodes i in [c*1024, (c+1)*1024).

Math (validated to rel 6.4e-3 vs the fp32 reference under the harness
metric):  the reference normalizes columns of M, thresholds at 1e-6,
aggregates the W_in-transformed features with a weighted mean, applies
W_out, residual, LN.  The threshold mask is numerically irrelevant
(validated), the column normalization cancels between numerator and weight
sum, and both W_in and W_out commute out of the aggregation, so with
    G = M^T @ [Xc | 1 | 0]   (per-core [1024, 258] from its column shard)
    xrb = X[shard] + (b_in @ W_out + b_out)
    out_i = LN(G[i,:256] + G[i,256]*xrb_i) * ln_scale + ln_bias
(LayerNorm is invariant to the positive per-row scale wsum, so no
reciprocal or PSUM evacuation pass is needed).

Schedule: the whole input of a core is packed on the host into ONE fp16
DRAM blob laid out in exact consumption order, and streamed by a single
sync-queue DMA program - a strictly sequential HBM walk measured at
~375-425 GB/s vs ~320 GB/s for the old multi-tensor stream that hopped
between m_z1/x_aug/m_z2/xrb regions.  Zone 1 (j-tiles 0..31) interleaves
each Xc tile with its M tile inside one segment, so one transfer per
chunk feeds the PE just-in-time and the first matmul starts ~10.5us in
(vs 15.3us for the old scalar-queue opening).  Zone 2 is block-major:
phase A (j-tiles 32..47) and phase B (48..63) for blocks 0..6, then all
32 zone-2 j-tiles of block 7 last, so the next-to-last block closes
3.5us before the final matmul and the DVE epilogue backlog drains
before block 7's exposed tail chain.  Zone-2 Xc tiles and the xrb
residual (fp32 bit-cast into the fp16 blob) ride the zone-2 DMA slack.
Every tile is SBUF-resident (~184 KB/partition), so the stream is never
throttled by pool-buffer rotation.  Epilogues overlap the next block's
matmuls (rstd via one ACT Rsqrt, no reciprocal hop); stores ride the
scalar HWDGE queue (the old gpsimd SWDGE stores forced a ~2us drain
into the end-of-program ceremony).  Output is fp16, upcast on host.
"""

import numpy as np

import concourse.bass as bass
import concourse.mybir as mybir
import concourse.tile as tile
from concourse import bacc
from concourse.bass import ts
from concourse.bass_utils import run_bass_kernel_spmd

F32 = mybir.dt.float32
F16 = mybir.dt.float16
AF = mybir.ActivationFunctionType

N, D, NCORES = 8192, 256, 8
P = 128
LN_EPS = 1e-5

Z1 = 32              # zone-1 j-tiles (supertile-major, xa|m interleaved)
ZH = 16              # zone-2 phase length in j-tiles (blocks 0..6)
DAUG = D + 2         # [Xc | 1 | 0]
SEG = DAUG + 1024    # zone-1 per-j-tile segment cols (xa 258 | m 1024)
NIB = 8              # output row-blocks per core
NB7 = NIB - 1        # blocks handled in phases A/B

# zone-1 chunk boundaries (in j-tiles): small opening chunks so the DMA
# issue pipeline and the PE warm up together, then quads
Z1B = [0, 1, 2, 4, 8, 12, 16, 20, 24, 28, 32]

# blob column offsets (fp16 cols per partition), in stream order:
# [z1 | xa2a | z2a b0..b3 | xrb | z2a b4..b6 | xa2b | z2b b0..b6 | z2c b7]
OFF_XA2A = Z1 * SEG
OFF_Z2A0 = OFF_XA2A + ZH * DAUG           # z2a blocks 0..3
OFF_XRB = OFF_Z2A0 + 4 * ZH * P           # xrb fp32 bitcast (4096 f16 cols)
OFF_Z2A4 = OFF_XRB + 2 * NIB * D          # z2a blocks 4..6
OFF_XA2B = OFF_Z2A4 + 3 * ZH * P
OFF_Z2B = OFF_XA2B + ZH * DAUG            # z2b blocks 0..6
OFF_Z2C = OFF_Z2B + NB7 * ZH * P          # block 7, all 32 zone-2 j-tiles
TOT = OFF_Z2C + 2 * ZH * P


def build_program(ln_affine=False):
    nc = bacc.Bacc("TRN2", target_bir_lowering=False, debug=False,
                   num_devices=NCORES)
    blob = nc.dram_tensor("blob", [P, TOT], F16, kind="ExternalInput")
    ln_s = nc.dram_tensor("ln_s", [1, D], F32, kind="ExternalInput")
    ln_b = nc.dram_tensor("ln_b", [1, D], F32, kind="ExternalInput")
    out = nc.dram_tensor("out_shard", [P, NIB * D], F16,
                         kind="ExternalOutput")

    with tile.TileContext(nc) as tc:
        with (
            tc.tile_pool(name="const", bufs=1) as const,
            tc.tile_pool(name="work", bufs=1) as work,
            tc.tile_pool(name="pp", bufs=1, space="PSUM") as pp,
        ):
            # small constants first (ACT Rsqrt table loads at t~0 instead
            # of stalling the first epilogue)
            eps_t = const.tile([P, 1], F32)
            nc.vector.memset(eps_t[:], LN_EPS)
            warm = const.tile([P, 2], F32)
            nc.scalar.activation(warm[:], eps_t[:].to_broadcast((P, 2)),
                                 AF.Abs_reciprocal_sqrt, bias=eps_t[:], scale=1.0)
            if ln_affine:
                lns_bc = const.tile([P, D], F32)
                nc.scalar.dma_start(lns_bc[:], ln_s[:].to_broadcast((P, D)))
                lnb_bc = const.tile([P, D], F32)
                nc.scalar.dma_start(lnb_bc[:], ln_b[:].to_broadcast((P, D)))

            # ---- the single paced DMA stream, in blob order ----
            z1c = []
            for ci in range(len(Z1B) - 1):
                lo, hi = Z1B[ci], Z1B[ci + 1]
                t = const.tile([P, hi - lo, SEG], F16, name=f"z1c{ci}")
                nc.sync.dma_start(t[:], blob[:, lo * SEG:hi * SEG])
                z1c.append(t)
            z1of = []
            for ci in range(len(Z1B) - 1):
                z1of += [(ci, jt - Z1B[ci])
                         for jt in range(Z1B[ci], Z1B[ci + 1])]

            xa2a = const.tile([P, ZH, DAUG], F16, name="xa2a")
            nc.sync.dma_start(xa2a[:], blob[:, OFF_XA2A:OFF_XA2A + ZH * DAUG])
            z2a = []
            for b in range(4):
                t = const.tile([P, ZH, P], F16, name=f"z2a{b}")
                o = OFF_Z2A0 + b * ZH * P
                nc.sync.dma_start(t[:], blob[:, o:o + ZH * P])
                z2a.append(t)
            xrb = const.tile([P, NIB, D], F32, name="xrb")
            nc.sync.dma_start(
                xrb[:], blob[:, OFF_XRB:OFF_XRB + 2 * NIB * D].bitcast(F32))
            for b in range(4, NB7):
                t = const.tile([P, ZH, P], F16, name=f"z2a{b}")
                o = OFF_Z2A4 + (b - 4) * ZH * P
                nc.sync.dma_start(t[:], blob[:, o:o + ZH * P])
                z2a.append(t)
            xa2b = const.tile([P, ZH, DAUG], F16, name="xa2b")
            nc.sync.dma_start(xa2b[:], blob[:, OFF_XA2B:OFF_XA2B + ZH * DAUG])
            z2b = []
            for b in range(NB7):
                t = const.tile([P, ZH, P], F16, name=f"z2b{b}")
                o = OFF_Z2B + b * ZH * P
                nc.sync.dma_start(t[:], blob[:, o:o + ZH * P])
                z2b.append(t)
            # block 7's chunk split in two so the PE overlaps the last
            # arrivals on DMA-slow runs instead of waiting for one big sem
            z2c = const.tile([P, 2 * ZH, P], F16, name="z2c")
            nc.sync.dma_start(z2c[:, 0:ZH, :],
                              blob[:, OFF_Z2C:OFF_Z2C + ZH * P])
            nc.sync.dma_start(z2c[:, ZH:2 * ZH, :],
                              blob[:, OFF_Z2C + ZH * P:OFF_Z2C + 2 * ZH * P])

            # ---- matmuls: G[b] += M_tile^T @ Xc_aug[jt] ----
            g = [pp.tile([P, DAUG], F32, tag=f"g{b}", name=f"g{b}")
                 for b in range(NIB)]

            def epilogue(b):
                # LayerNorm is invariant to positive per-row scaling:
                # normalize y2 = G + wsum*xrb directly - no reciprocal.
                y = work.tile([P, D], F32, name=f"y{b}")
                nc.vector.affine_then_add(y[:], xrb[:, b, :], g[b][:, 0:D],
                                          g[b][:, D:D + 1], 0.0)
                st6 = work.tile([P, 6], F32, name=f"st6_{b}")
                nc.vector.bn_stats(st6[:], y[:])
                mv = work.tile([P, 2], F32, name=f"mv{b}")
                nc.vector.bn_aggr(mv[:], st6[:])
                # rstd = 1/sqrt(var + eps) in one ACT op; the rsqrt table
                # error is multiplicative on the row, which the rel metric
                # tolerates (validated: rel unchanged at 6.4e-3 scale)
                rstd = work.tile([P, 1], F32, name=f"rstd{b}")
                nc.scalar.activation(rstd[:], mv[:, 1:2],
                                     AF.Abs_reciprocal_sqrt,
                                     bias=eps_t[:], scale=1.0)
                yn = work.tile([P, D], F32 if ln_affine else F16,
                               name=f"yn{b}")
                if b == NIB - 1:
                    # final block: one fused (y - mu) * rstd on DVE - the
                    # shortest exposed tail chain (no bln hop, no ACT dep)
                    nc.vector.tensor_scalar(yn[:], y[:], mv[:, 0:1], rstd[:],
                                            op0=mybir.AluOpType.subtract,
                                            op1=mybir.AluOpType.mult)
                elif b % 2 == 0:   # split normalize across ACT and DVE
                    bln = work.tile([P, 1], F32, name=f"bln{b}")
                    nc.vector.scalar_tensor_tensor(
                        bln[:], in0=mv[:, 0:1], scalar=-1.0, in1=rstd[:],
                        op0=mybir.AluOpType.mult, op1=mybir.AluOpType.mult)
                    nc.scalar.activation(yn[:], y[:], AF.Identity,
                                         bias=bln[:], scale=rstd[:])
                else:
                    nc.vector.tensor_scalar(yn[:], y[:], mv[:, 0:1], rstd[:],
                                            op0=mybir.AluOpType.subtract,
                                            op1=mybir.AluOpType.mult)
                res = yn
                if ln_affine:
                    t1 = work.tile([P, D], F32, name=f"aff{b}")
                    nc.vector.tensor_mul(t1[:], yn[:], lns_bc[:])
                    res = work.tile([P, D], F16, name=f"aff2_{b}")
                    nc.vector.tensor_add(res[:], t1[:], lnb_bc[:])
                nc.scalar.dma_start(out[:, b * D:(b + 1) * D], res[:])

            for jt in range(Z1):
                ci, t = z1of[jt]
                for b in range(NIB):
                    nc.tensor.matmul(
                        g[b][:],
                        lhsT=z1c[ci][:, t, DAUG + b * P:DAUG + (b + 1) * P],
                        rhs=z1c[ci][:, t, 0:DAUG],
                        start=(jt == 0), stop=False)
            for b in range(NB7):
                for t in range(ZH):
                    nc.tensor.matmul(g[b][:], lhsT=z2a[b][:, t, :],
                                     rhs=xa2a[:, t, :],
                                     start=False, stop=False)
            for b in range(NB7):
                for t in range(ZH):
                    nc.tensor.matmul(g[b][:], lhsT=z2b[b][:, t, :],
                                     rhs=xa2b[:, t, :],
                                     start=False, stop=(t == ZH - 1))
                epilogue(b)
            b7 = NIB - 1
            for t in range(2 * ZH):
                rhs = xa2a[:, t, :] if t < ZH else xa2b[:, t - ZH, :]
                nc.tensor.matmul(g[b7][:], lhsT=z2c[:, t, :], rhs=rhs,
                                 start=False, stop=(t == 2 * ZH - 1))
            epilogue(b7)

    nc.compile()
    return nc


_cache = {}


def _get_program(ln_affine):
    if ln_affine not in _cache:
        _cache[ln_affine] = build_program(ln_affine=ln_affine)
    return _cache[ln_affine]


def _pack(a, blocks, row_len):
    """[blocks*128, row_len] -> [128, blocks*row_len] with logical row
    blk*128+p at (p, blk*row_len)."""
    return np.ascontiguousarray(
        a.reshape(blocks, P, row_len).transpose(1, 0, 2).reshape(
            P, blocks * row_len))


def prepare_inputs(node_features, mobility_matrix, W_in, b_in, W_out, b_out,
                   ln_scale, ln_bias):
    x = np.asarray(node_features, dtype=np.float32)
    m16 = np.asarray(mobility_matrix, dtype=np.float16)
    w_in = np.asarray(W_in, dtype=np.float64)
    b_in_ = np.asarray(b_in, dtype=np.float64)
    w_out = np.asarray(W_out, dtype=np.float64)
    b_out_ = np.asarray(b_out, dtype=np.float64)
    lns = np.asarray(ln_scale, dtype=np.float32)
    lnb = np.asarray(ln_bias, dtype=np.float32)

    w_c = (w_in @ w_out).astype(np.float32)
    bias_c = (b_in_ @ w_out + b_out_).astype(np.float32)

    s = N // NCORES
    ln_affine = not (np.all(lns == 1.0) and np.all(lnb == 0.0))

    xc = x @ w_c
    x_aug = np.zeros((N, DAUG), dtype=np.float16)
    x_aug[:, :D] = xc
    x_aug[:, D] = 1.0
    xa_r = x_aug.reshape(N // P, P, DAUG)          # [jt, p, 258]

    in_maps = []
    for c in range(NCORES):
        msh = m16[:, c * s:(c + 1) * s]
        m_r = msh.reshape(N // P, P, s)            # [jt, p, 1024]
        # zone 1: [xa_jt | m_jt] interleaved, jt-major
        z1 = np.concatenate([xa_r[:Z1], m_r[:Z1]], axis=2)   # [32, p, 1282]
        z1_blob = z1.transpose(1, 0, 2).reshape(P, Z1 * SEG)
        # zone-2 xa halves: [p, jt, 258] -> cols
        xa2a = xa_r[Z1:Z1 + ZH].transpose(1, 0, 2).reshape(P, ZH * DAUG)
        xa2b = xa_r[Z1 + ZH:].transpose(1, 0, 2).reshape(P, ZH * DAUG)
        # zone-2 M, per block: [b][p][t*128+f]
        z2 = m_r[Z1:].reshape(2 * ZH, P, NIB, P)
        z2a = z2[:ZH, :, :NB7].transpose(2, 1, 0, 3).reshape(NB7, P, ZH * P)
        z2bh = z2[ZH:, :, :NB7].transpose(2, 1, 0, 3).reshape(NB7, P, ZH * P)
        z2c = z2[:, :, NB7].transpose(1, 0, 2).reshape(P, 2 * ZH * P)
        xrb = _pack(x[c * s:(c + 1) * s] + bias_c, s // P, D)  # [128,2048]f32
        xrb16 = np.ascontiguousarray(xrb).view(np.float16)     # [128, 4096]
        blob = np.concatenate(
            [z1_blob, xa2a] + list(z2a[:4]) + [xrb16] + list(z2a[4:])
            + [xa2b] + list(z2bh) + [z2c], axis=1)
        assert blob.shape == (P, TOT), blob.shape
        in_maps.append({
            "blob": np.ascontiguousarray(blob),
            "ln_s": lns.reshape(1, D),
            "ln_b": lnb.reshape(1, D),
        })
    return in_maps, ln_affine


def run(in_maps, ln_affine, **kwargs):
    nc = _get_program(ln_affine)
    return run_bass_kernel_spmd(nc, in_maps, core_ids=list(range(NCORES)),
                                **kwargs)


def unpack_output(res) -> np.ndarray:
    outs = []
    for c in range(NCORES):
        o = res.results[c]["out_shard"]
        outs.append(o.reshape(P, N // NCORES // P, D).transpose(1, 0, 2)
                    .reshape(N // NCORES, D).astype(np.float32))
    return np.concatenate(outs, axis=0)


def kernel(**inputs) -> np.ndarray:
    in_maps, ln_affine = prepare_inputs(**inputs)
    return unpack_output(run(in_maps, ln_affine))


# revision 11
# speedup vs baseline: 1.0452x; 1.0452x over previous
"""MobilityGNNLayer Trainium2 kernel (8 NeuronCores, SPMD, no collectives).

Sharding: 1D partition of the destination axis (columns of mobility_matrix).
Core c owns destination nodes i in [c*1024, (c+1)*1024).

Math (validated to rel 6.4e-3 vs the fp32 reference under the harness
metric):  the reference normalizes columns of M, thresholds at 1e-6,
aggregates the W_in-transformed features with a weighted mean, applies
W_out, residual, LN.  The threshold mask is numerically irrelevant
(validated), the column normalization cancels between numerator and weight
sum, and both W_in and W_out commute out of the aggregation, so with
    G = M^T @ [Xc | 1 | 0]   (per-core [1024, 258] from its column shard)
    xrb = X[shard] + (b_in @ W_out + b_out)
    out_i = LN(G[i,:256] + G[i,256]*xrb_i) * ln_scale + ln_bias
(LayerNorm is invariant to the positive per-row scale wsum, so no
reciprocal or PSUM evacuation pass is needed).

Schedule: the whole input of a core is packed on the host into ONE fp16
DRAM blob laid out in exact consumption order, and streamed by a single
sync-queue DMA program - a strictly sequential HBM walk measured at
~375-425 GB/s vs ~320 GB/s for the old multi-tensor stream that hopped
between m_z1/x_aug/m_z2/xrb regions.  Zone 1 (j-tiles 0..31) interleaves
each Xc tile with its M tile inside one segment, so one transfer per
chunk feeds the PE just-in-time and the first matmul starts ~10.5us in
(vs 15.3us for the old scalar-queue opening).  Zone 2 is block-major:
phase A (j-tiles 32..47) and phase B (48..63) for blocks 0..6, then all
32 zone-2 j-tiles of block 7 last, so the next-to-last block closes
3.5us before the final matmul and the DVE epilogue backlog drains
before block 7's exposed tail chain.  Zone-2 Xc tiles and the xrb
residual (fp32 bit-cast into the fp16 blob) ride the zone-2 DMA slack.
Every tile is SBUF-resident (~184 KB/partition), so the stream is never
throttled by pool-buffer rotation.  Epilogues overlap the next block's
matmuls (rstd via one ACT Rsqrt, no reciprocal hop); stores ride the
scalar HWDGE queue (the old gpsimd SWDGE stores forced a ~2us drain
into the end-of-program ceremony).  Output is fp16, upcast on host.
"""

import numpy as np

import concourse.bass as bass
import concourse.mybir as mybir
import concourse.tile as tile
from concourse import bacc
from concourse.bass import ts
from concourse.bass_utils import run_bass_kernel_spmd

F32 = mybir.dt.float32
F16 = mybir.dt.float16
AF = mybir.ActivationFunctionType

N, D, NCORES = 8192, 256, 8
P = 128
LN_EPS = 1e-5

Z1 = 32              # zone-1 j-tiles (supertile-major, xa|m interleaved)
ZH = 16              # zone-2 phase length in j-tiles (blocks 0..6)
DAUG = D + 2         # [Xc | 1 | 0]
SEG = DAUG + 1024    # zone-1 per-j-tile segment cols (xa 258 | m 1024)
NIB = 8              # output row-blocks per core
NB7 = NIB - 1        # blocks handled in phases A/B

# zone-1 chunk boundaries (in j-tiles): small opening chunks so the DMA
# issue pipeline and the PE warm up together, then quads
Z1B = [0, 1, 2, 4, 8, 12, 16, 20, 24, 28, 32]

# blob column offsets (fp16 cols per partition), in stream order:
# [z1 | xa2a | z2a b0..b3 | xrb | z2a b4..b6 | xa2b | z2b b0..b6 | z2c b7]
OFF_XA2A = Z1 * SEG
OFF_Z2A0 = OFF_XA2A + ZH * DAUG           # z2a blocks 0..3
OFF_XRB = OFF_Z2A0 + 4 * ZH * P           # xrb fp32 bitcast (4096 f16 cols)
OFF_Z2A4 = OFF_XRB + 2 * NIB * D          # z2a blocks 4..6
OFF_XA2B = OFF_Z2A4 + 3 * ZH * P
OFF_Z2B = OFF_XA2B + ZH * DAUG            # z2b blocks 0..6
OFF_Z2C = OFF_Z2B + NB7 * ZH * P          # block 7, all 32 zone-2 j-tiles
TOT = OFF_Z2C + 2 * ZH * P


def build_program(ln_affine=False):
    nc = bacc.Bacc("TRN2", target_bir_lowering=False, debug=False,
                   num_devices=NCORES)
    blob = nc.dram_tensor("blob", [P, TOT], F16, kind="ExternalInput")
    ln_s = nc.dram_tensor("ln_s", [1, D], F32, kind="ExternalInput")
    ln_b = nc.dram_tensor("ln_b", [1, D], F32, kind="ExternalInput")
    out = nc.dram_tensor("out_shard", [P, NIB * D], F16,
                         kind="ExternalOutput")

    with tile.TileContext(nc) as tc:
        with (
            tc.tile_pool(name="const", bufs=1) as const,
            tc.tile_pool(name="work", bufs=1) as work,
            tc.tile_pool(name="pp", bufs=1, space="PSUM") as pp,
        ):
            # small constants first (ACT Rsqrt table loads at t~0 instead
            # of stalling the first epilogue)
            eps_t = const.tile([P, 1], F32)
            nc.vector.memset(eps_t[:], LN_EPS)
            warm = const.tile([P, 2], F32)
            nc.scalar.activation(warm[:], eps_t[:].to_broadcast((P, 2)),
                                 AF.Abs_reciprocal_sqrt, bias=eps_t[:], scale=1.0)
            if ln_affine:
                lns_bc = const.tile([P, D], F32)
                nc.scalar.dma_start(lns_bc[:], ln_s[:].to_broadcast((P, D)))
                lnb_bc = const.tile([P, D], F32)
                nc.scalar.dma_start(lnb_bc[:], ln_b[:].to_broadcast((P, D)))

            # ---- the single paced DMA stream, in blob order ----
            z1c = []
            for ci in range(len(Z1B) - 1):
                lo, hi = Z1B[ci], Z1B[ci + 1]
                t = const.tile([P, hi - lo, SEG], F16, name=f"z1c{ci}")
                nc.sync.dma_start(t[:], blob[:, lo * SEG:hi * SEG])
                z1c.append(t)
            z1of = []
            for ci in range(len(Z1B) - 1):
                z1of += [(ci, jt - Z1B[ci])
                         for jt in range(Z1B[ci], Z1B[ci + 1])]

            xa2a = const.tile([P, ZH, DAUG], F16, name="xa2a")
            nc.sync.dma_start(xa2a[:], blob[:, OFF_XA2A:OFF_XA2A + ZH * DAUG])
            z2a = []
            for b in range(4):
                t = const.tile([P, ZH, P], F16, name=f"z2a{b}")
                o = OFF_Z2A0 + b * ZH * P
                nc.sync.dma_start(t[:], blob[:, o:o + ZH * P])
                z2a.append(t)
            xrb = const.tile([P, NIB, D], F32, name="xrb")
            nc.sync.dma_start(
                xrb[:], blob[:, OFF_XRB:OFF_XRB + 2 * NIB * D].bitcast(F32))
            for b in range(4, NB7):
                t = const.tile([P, ZH, P], F16, name=f"z2a{b}")
                o = OFF_Z2A4 + (b - 4) * ZH * P
                nc.sync.dma_start(t[:], blob[:, o:o + ZH * P])
                z2a.append(t)
            xa2b = const.tile([P, ZH, DAUG], F16, name="xa2b")
            nc.sync.dma_start(xa2b[:], blob[:, OFF_XA2B:OFF_XA2B + ZH * DAUG])
            z2b = []
            for b in range(NB7):
                t = const.tile([P, ZH, P], F16, name=f"z2b{b}")
                o = OFF_Z2B + b * ZH * P
                nc.sync.dma_start(t[:], blob[:, o:o + ZH * P])
                z2b.append(t)
            # block 7's chunk split in two so the PE overlaps the last
            # arrivals on DMA-slow runs instead of waiting for one big sem
            z2c = const.tile([P, 2 * ZH, P], F16, name="z2c")
            nc.sync.dma_start(z2c[:, 0:ZH, :],
                              blob[:, OFF_Z2C:OFF_Z2C + ZH * P])
            nc.sync.dma_start(z2c[:, ZH:2 * ZH, :],
                              blob[:, OFF_Z2C + ZH * P:OFF_Z2C + 2 * ZH * P])

            # ---- matmuls: G[b] += M_tile^T @ Xc_aug[jt] ----
            g = [pp.tile([P, DAUG], F32, tag=f"g{b}", name=f"g{b}")
                 for b in range(NIB)]

            def epilogue(b):
                # LayerNorm is invariant to positive per-row scaling:
                # normalize y2 = G + wsum*xrb directly - no reciprocal.
                y = work.tile([P, D], F32, name=f"y{b}")
                nc.vector.affine_then_add(y[:], xrb[:, b, :], g[b][:, 0:D],
                                          g[b][:, D:D + 1], 0.0)
                st6 = work.tile([P, 6], F32, name=f"st6_{b}")
                nc.vector.bn_stats(st6[:], y[:])
                mv = work.tile([P, 2], F32, name=f"mv{b}")
                nc.vector.bn_aggr(mv[:], st6[:])
                # rstd = 1/sqrt(var + eps) in one ACT op; the rsqrt table
                # error is multiplicative on the row, which the rel metric
                # tolerates (validated: rel unchanged at 6.4e-3 scale)
                rstd = work.tile([P, 1], F32, name=f"rstd{b}")
                nc.scalar.activation(rstd[:], mv[:, 1:2],
                                     AF.Abs_reciprocal_sqrt,
                                     bias=eps_t[:], scale=1.0)
                yn = work.tile([P, D], F32 if ln_affine else F16,
                               name=f"yn{b}")
                if b == NIB - 1:
                    # final block: one fused (y - mu) * rstd on DVE - the
                    # shortest exposed tail chain (no bln hop, no ACT dep)
                    nc.vector.tensor_scalar(yn[:], y[:], mv[:, 0:1], rstd[:],
                                            op0=mybir.AluOpType.subtract,
                                            op1=mybir.AluOpType.mult)
                elif b % 2 == 0:   # split normalize across ACT and DVE
                    bln = work.tile([P, 1], F32, name=f"bln{b}")
                    nc.vector.scalar_tensor_tensor(
                        bln[:], in0=mv[:, 0:1], scalar=-1.0, in1=rstd[:],
                        op0=mybir.AluOpType.mult, op1=mybir.AluOpType.mult)
                    nc.scalar.activation(yn[:], y[:], AF.Identity,
                                         bias=bln[:], scale=rstd[:])
                else:
                    nc.vector.tensor_scalar(yn[:], y[:], mv[:, 0:1], rstd[:],
                                            op0=mybir.AluOpType.subtract,
                                            op1=mybir.AluOpType.mult)
                res = yn
                if ln_affine:
                    t1 = work.tile([P, D], F32, name=f"aff{b}")
                    nc.vector.tensor_mul(t1[:], yn[:], lns_bc[:])
                    res = work.tile([P, D], F16, name=f"aff2_{b}")
                    nc.vector.tensor_add(res[:], t1[:], lnb_bc[:])
                if b == NIB - 1:
                    # final store split across the two idle HWDGE queues so
                    # the issue latencies overlap on the exposed tail
                    h = D // 2
                    nc.scalar.dma_start(out[:, b * D:b * D + h], res[:, 0:h])
                    nc.sync.dma_start(out[:, b * D + h:(b + 1) * D],
                                      res[:, h:D])
                else:
                    nc.scalar.dma_start(out[:, b * D:(b + 1) * D], res[:])

            for jt in range(Z1):
                ci, t = z1of[jt]
                for b in range(NIB):
                    nc.tensor.matmul(
                        g[b][:],
                        lhsT=z1c[ci][:, t, DAUG + b * P:DAUG + (b + 1) * P],
                        rhs=z1c[ci][:, t, 0:DAUG],
                        start=(jt == 0), stop=False)
            for b in range(NB7):
                for t in range(ZH):
                    nc.tensor.matmul(g[b][:], lhsT=z2a[b][:, t, :],
                                     rhs=xa2a[:, t, :],
                                     start=False, stop=False)
            for b in range(NB7):
                for t in range(ZH):
                    nc.tensor.matmul(g[b][:], lhsT=z2b[b][:, t, :],
                                     rhs=xa2b[:, t, :],
                                     start=False, stop=(t == ZH - 1))
                epilogue(b)
            b7 = NIB - 1
            for t in range(2 * ZH):
                rhs = xa2a[:, t, :] if t < ZH else xa2b[:, t - ZH, :]
                nc.tensor.matmul(g[b7][:], lhsT=z2c[:, t, :], rhs=rhs,
                                 start=False, stop=(t == 2 * ZH - 1))
            epilogue(b7)

    nc.compile()
    return nc


_cache = {}


def _get_program(ln_affine):
    if ln_affine not in _cache:
        _cache[ln_affine] = build_program(ln_affine=ln_affine)
    return _cache[ln_affine]


def _pack(a, blocks, row_len):
    """[blocks*128, row_len] -> [128, blocks*row_len] with logical row
    blk*128+p at (p, blk*row_len)."""
    return np.ascontiguousarray(
        a.reshape(blocks, P, row_len).transpose(1, 0, 2).reshape(
            P, blocks * row_len))


def prepare_inputs(node_features, mobility_matrix, W_in, b_in, W_out, b_out,
                   ln_scale, ln_bias):
    x = np.asarray(node_features, dtype=np.float32)
    m16 = np.asarray(mobility_matrix, dtype=np.float16)
    w_in = np.asarray(W_in, dtype=np.float64)
    b_in_ = np.asarray(b_in, dtype=np.float64)
    w_out = np.asarray(W_out, dtype=np.float64)
    b_out_ = np.asarray(b_out, dtype=np.float64)
    lns = np.asarray(ln_scale, dtype=np.float32)
    lnb = np.asarray(ln_bias, dtype=np.float32)

    w_c = (w_in @ w_out).astype(np.float32)
    bias_c = (b_in_ @ w_out + b_out_).astype(np.float32)

    s = N // NCORES
    ln_affine = not (np.all(lns == 1.0) and np.all(lnb == 0.0))

    xc = x @ w_c
    x_aug = np.zeros((N, DAUG), dtype=np.float16)
    x_aug[:, :D] = xc
    x_aug[:, D] = 1.0
    xa_r = x_aug.reshape(N // P, P, DAUG)          # [jt, p, 258]

    in_maps = []
    for c in range(NCORES):
        msh = m16[:, c * s:(c + 1) * s]
        m_r = msh.reshape(N // P, P, s)            # [jt, p, 1024]
        # zone 1: [xa_jt | m_jt] interleaved, jt-major
        z1 = np.concatenate([xa_r[:Z1], m_r[:Z1]], axis=2)   # [32, p, 1282]
        z1_blob = z1.transpose(1, 0, 2).reshape(P, Z1 * SEG)
        # zone-2 xa halves: [p, jt, 258] -> cols
        xa2a = xa_r[Z1:Z1 + ZH].transpose(1, 0, 2).reshape(P, ZH * DAUG)
        xa2b = xa_r[Z1 + ZH:].transpose(1, 0, 2).reshape(P, ZH * DAUG)
        # zone-2 M, per block: [b][p][t*128+f]
        z2 = m_r[Z1:].reshape(2 * ZH, P, NIB, P)
        z2a = z2[:ZH, :, :NB7].transpose(2, 1, 0, 3).reshape(NB7, P, ZH * P)
        z2bh = z2[ZH:, :, :NB7].transpose(2, 1, 0, 3).reshape(NB7, P, ZH * P)
        z2c = z2[:, :, NB7].transpose(1, 0, 2).reshape(P, 2 * ZH * P)
        xrb = _pack(x[c * s:(c + 1) * s] + bias_c, s // P, D)  # [128,2048]f32
        xrb16 = np.ascontiguousarray(xrb).view(np.float16)     # [128, 4096]
        blob = np.concatenate(
            [z1_blob, xa2a] + list(z2a[:4]) + [xrb16] + list(z2a[4:])
            + [xa2b] + list(z2bh) + [z2c], axis=1)
        assert blob.shape == (P, TOT), blob.shape
        in_maps.append({
            "blob": np.ascontiguousarray(blob),
            "ln_s": lns.reshape(1, D),
            "ln_b": lnb.reshape(1, D),
        })
    return in_maps, ln_affine


def run(in_maps, ln_affine, **kwargs):
    nc = _get_program(ln_affine)
    return run_bass_kernel_spmd(nc, in_maps, core_ids=list(range(NCORES)),
                                **kwargs)


def unpack_output(res) -> np.ndarray:
    outs = []
    for c in range(NCORES):
        o = res.results[c]["out_shard"]
        outs.append(o.reshape(P, N // NCORES // P, D).transpose(1, 0, 2)
                    .reshape(N // NCORES, D).astype(np.float32))
    return np.concatenate(outs, axis=0)


def kernel(**inputs) -> np.ndarray:
    in_maps, ln_affine = prepare_inputs(**inputs)
    return unpack_output(run(in_maps, ln_affine))
